# revision 1
# baseline (speedup 1.0000x reference)
"""Multi-head attention (N=4, L=2048, E=1024, H=16, DK=64) on 8 TRN2 cores.

The reference splits heads with a PLAIN RESHAPE (n, l, H*DK) -> (n, H, l, DK),
so "head" h is really a contiguous block of 128 tokens, and the 2048 attention
positions inside it are (token, s) pairs where s indexes sixteen 64-wide
E-slices.  Per (batch, block):
    Qb = q[n, 128b:128b+128, :].reshape(2048, 64)   (same for K, V)
    out_block = softmax(Qb Kb^T / 8) Vb  -> reshape(128, E) -> rows of out
Positions are processed in permuted order p' = 128*s + tok (a permutation of
the softmax axis; unpermuted on the way out).

Sharding: core c owns token rows [n, 256c : 256c+256) for every batch n (two
128-token blocks per batch).  Outputs are disjoint rows; the host scatters.
Each core gets the full weights (bf16, all resident in SBUF) and only its own
x columns.

Matmuls in bf16 (fp32 PSUM accumulate).  Per-core flow, software-pipelined so
batch n's attention overlaps batch n+1's projections and batch n-1's
normalize/output-projection (the `feed` queue spreads that work across
attention units to keep both PE and ScalarE fed):
  x_sb [E, 1024 tok] resident.
  V:    V_nat [128 tok, E] per (n, B), evicted into per-s slices + ones col.
  Q/K:  [e_out 128, tok 256] PSUM tiles evicted straight into the permuted
        layout q1t/k1t [128 = 2B x 64 d, 2048 p'] (per-batch tiles).
  Attention per (n, u = q' chunk of 512): 16 key tiles; the two blocks'
        score matmuls are row-packed on the PE (disjoint 64-row groups, they
        run concurrently); exp on ScalarE over [128, 1024] PSUM (scale=1/8
        folded), bf16 out; PV accumulates [V|ones].T @ expS -> [65, 512] fp32
        (row 64 = softmax denominator); rows 0-63 evicted unnormalized into
        opT, denominator rows collected at 32-aligned partitions.
  Normalize per batch: two batched DVE reciprocals [128, 512], broadcast of
        1/denom to 128 partitions via a ones-column f32r matmul, in-place DVE
        multiply on opT.
  Out proj per (n, B): accumulate over 8 e_in tiles against resident woT.
"""

import ml_dtypes
import numpy as np

import concourse.bass as bass
import concourse.mybir as mybir
import concourse.tile as tile
from concourse import bacc
from concourse.bass_utils import run_bass_kernel_spmd

N, L, E, H = 4, 2048, 1024, 16
DK = E // H  # 64
NC = 8
BPC = 2  # token blocks per core per batch
TPB = 128  # tokens per block
TPN = BPC * TPB  # 256 tokens per batch per core
TC = N * TPN  # 1024 tokens per core
P = 128
QC = 512  # q' chunk
NQC = 2048 // QC  # 4
NKT = 2048 // P  # 16 key tiles (= s values)
ET = E // P  # 8

F32 = mybir.dt.float32
BF16 = mybir.dt.bfloat16
MM_DT = BF16


def build_nc():
    nc = bacc.Bacc("TRN2", target_bir_lowering=False, debug=False, num_devices=NC)

    xTc = nc.dram_tensor("xTc", [E, TC], MM_DT, kind="ExternalInput").ap()
    wqT = nc.dram_tensor("wqT", [E, E], MM_DT, kind="ExternalInput").ap()
    wkT = nc.dram_tensor("wkT", [E, E], MM_DT, kind="ExternalInput").ap()
    wvT = nc.dram_tensor("wvT", [E, E], MM_DT, kind="ExternalInput").ap()
    woT = nc.dram_tensor("woT", [E, E], MM_DT, kind="ExternalInput").ap()
    outp = nc.dram_tensor("outp", [TC, E], F32, kind="ExternalOutput").ap()

    with tile.TileContext(nc) as tc:
        with (
            tc.tile_pool(name="const", bufs=1) as const,
            tc.tile_pool(name="wpool", bufs=1) as wpool,
            tc.tile_pool(name="xv", bufs=2) as xv_pool,
            tc.tile_pool(name="qk1", bufs=2) as qk1_pool,
            tc.tile_pool(name="expp", bufs=6) as exp_pool,
            tc.tile_pool(name="opt", bufs=2) as opt_pool,
            tc.tile_pool(name="nrm", bufs=2) as nrm_pool,
            tc.tile_pool(name="ops", bufs=2) as op_pool,
            tc.tile_pool(name="scps", bufs=2, space="PSUM") as sc_psum,
            tc.tile_pool(name="pvps", bufs=4, space="PSUM") as pv_psum,
        ):
            ones_f32 = const.tile([P, P], F32)
            nc.vector.memset(ones_f32[:], 1.0)
            ones_r = const.tile([P, P], mybir.dt.float32r)
            nc.vector.tensor_copy(ones_r[:], ones_f32[:])

            # ---- resident x ----
            x_sb = xv_pool.tile([P, ET, TC], MM_DT, tag="xv", name="x_sb")
            xr_ = xTc.rearrange("(a p) t -> p a t", p=P)
            for n_ in range(N):
                nc.sync.dma_start(
                    out=x_sb[:, :, n_ * TPN : (n_ + 1) * TPN],
                    in_=xr_[:, :, n_ * TPN : (n_ + 1) * TPN],
                )

            def load_w(w_dram, nm):
                w_sb = wpool.tile([P, ET, E], MM_DT, tag=nm, name=nm)
                nc.sync.dma_start(
                    out=w_sb[:], in_=w_dram.rearrange("(a p) d -> p a d", p=P)
                )
                return w_sb

            wk_sb = load_w(wkT, "wk")
            wq_sb = load_w(wqT, "wq")
            wv_sb = load_w(wvT, "wv")
            wo_sb = load_w(woT, "wo")

            def project_batch(n):
                """Returns (tiles, [chunk emitters]) so projection work for
                batch n can be interleaved into batch n-1's attention units."""
                v_sb = qk1_pool.tile(
                    [P, BPC, NKT, DK + 1], MM_DT, tag="v", name="v_sb"
                )
                q1t = qk1_pool.tile([P, 2048], MM_DT, tag="q1", name="q1t")
                k1t = qk1_pool.tile([P, 2048], MM_DT, tag="k1", name="k1t")
                groups = []
                # K and Q: [e_out 128, tok 256] psums -> permuted q1t/k1t
                def qk_group(w_sb, dst, a2):
                    def emit():
                        ps = pv_psum.tile([P, TPN], F32, tag="pv", name="qkps")
                        for a in range(ET):
                            nc.tensor.matmul(
                                ps[:],
                                w_sb[:, a, a2 * P : (a2 + 1) * P],
                                x_sb[:, a, n * TPN : (n + 1) * TPN],
                                start=(a == 0),
                                stop=(a == ET - 1),
                            )
                        for sg in range(2):
                            s = a2 * 2 + sg
                            for B in range(BPC):
                                nc.vector.tensor_copy(
                                    dst[B * DK : (B + 1) * DK,
                                        s * TPB : (s + 1) * TPB],
                                    ps[sg * DK : (sg + 1) * DK,
                                       B * TPB : (B + 1) * TPB],
                                )
                    return emit

                def v_group(B, eh):
                    def emit():
                        tok0 = n * TPN + B * TPB
                        ps = pv_psum.tile([P, 512], F32, tag="pv", name="vps")
                        for a in range(ET):
                            nc.tensor.matmul(
                                ps[:],
                                x_sb[:, a, tok0 : tok0 + TPB],
                                wv_sb[:, a, eh * 512 : (eh + 1) * 512],
                                start=(a == 0),
                                stop=(a == ET - 1),
                            )
                        nc.vector.tensor_copy(
                            v_sb[:, B, eh * 8 : (eh + 1) * 8, 0:DK],
                            ps.rearrange("p (s d) -> p s d", d=DK),
                        )
                    return emit

                def ones_group():
                    nc.vector.tensor_copy(
                        v_sb[:, :, :, DK], ones_f32[:, 0 : BPC * NKT]
                    )

                for a2 in range(ET):
                    groups.append(qk_group(wk_sb, k1t, a2))
                for a2 in range(ET):
                    groups.append(qk_group(wq_sb, q1t, a2))
                for B in range(BPC):
                    for eh in range(2):
                        groups.append(v_group(B, eh))
                groups.append(ones_group)
                return (v_sb, q1t, k1t), groups

            # ---- per batch: project, attend, normalize, out-project ----
            # `feed` holds deferred fine-grained work (next batch's projection
            # chunks, previous batch's normalize/out-proj pieces) drained one
            # item per key-tile so the PE stream never starves ScalarE.
            tiles, groups = project_batch(0)
            for g in groups:
                g()
            next_state = None
            feed = []

            def make_normalize_piece(opT, rec, B, u):
                def emit():
                    r_ = B * NQC + u
                    rp = 32 * (r_ % 4)
                    bcp = pv_psum.tile([P, QC], F32, tag="pv", name="bcp")
                    nc.tensor.matmul(
                        bcp[:],
                        ones_r[rp : rp + 1, :],
                        rec[r_ // 4][rp : rp + 1, :],
                        start=True,
                        stop=True,
                        tile_position=(rp, 0),
                    )
                    for sg in range(2):
                        tgt = opT[sg * DK : (sg + 1) * DK,
                                  2 * u : 2 * u + 2, B, :]
                        nc.vector.tensor_mul(
                            tgt,
                            tgt,
                            bcp[sg * DK : (sg + 1) * DK, :].rearrange(
                                "d (sp t) -> d sp t", t=TPB
                            )[:, sg::2, :],
                        )
                return emit

            def make_outproj_piece(opT, n, B, half):
                def emit():
                    ps = pv_psum.tile([P, 512], F32, tag="pv", name="opps")
                    for a2 in range(ET):
                        nc.tensor.matmul(
                            ps[:],
                            opT[:, a2, B, :],
                            wo_sb[:, a2, half * 512 : (half + 1) * 512],
                            start=(a2 == 0),
                            stop=(a2 == ET - 1),
                        )
                    op_sb = op_pool.tile([P, 512], F32, tag="op")
                    nc.vector.tensor_copy(op_sb[:], ps[:])
                    r0 = n * TPN + B * TPB
                    nc.sync.dma_start(
                        out=outp[r0 : r0 + TPB, half * 512 : (half + 1) * 512],
                        in_=op_sb[:],
                    )
                return emit

            for n in range(N):
                while feed:
                    feed.pop(0)()  # safety drain before slot-reusing allocs
                v_sb, q1t, k1t = tiles
                if n + 1 < N:
                    next_state = project_batch(n + 1)
                    feed.extend(next_state[1])
                opT = opt_pool.tile([P, ET, BPC, TPB], MM_DT, tag="opT", name="opT")
                # denominator rows live at 32-aligned partitions of two tiles
                sums = [
                    nrm_pool.tile([P, QC], F32, tag="sums", name=f"sums{_i}")
                    for _i in range(2)
                ]
                for u in range(NQC):
                    # drain deferred work in small lumps at unit boundaries
                    take = (len(feed) + NQC - 1 - u) // (NQC - u) if feed else 0
                    for _ in range(min(take, len(feed))):
                        feed.pop(0)()
                    qsl = slice(u * QC, (u + 1) * QC)
                    pv = [
                        pv_psum.tile([DK + 1, QC], F32, tag="pv", name=f"pv{_b}")
                        for _b in range(BPC)
                    ]
                    for j in range(NKT):
                        sc = sc_psum.tile([P, BPC, QC], F32, tag="sc")
                        ksl = slice(j * TPB, (j + 1) * TPB)
                        for B in range(BPC):
                            bsl = slice(B * DK, (B + 1) * DK)
                            nc.tensor.matmul(
                                sc[:, B, :],
                                k1t[bsl, ksl],
                                q1t[bsl, qsl],
                                start=True,
                                stop=True,
                            )
                        exps = exp_pool.tile([P, BPC, QC], MM_DT, tag="exps")
                        nc.scalar.activation(
                            exps[:],
                            sc[:],
                            mybir.ActivationFunctionType.Exp,
                            scale=1.0 / np.sqrt(DK),
                        )
                        for B in range(BPC):
                            nc.tensor.matmul(
                                pv[B][:],
                                v_sb[:, B, j, :],
                                exps[:, B, :],
                                start=(j == 0),
                                stop=(j == NKT - 1),
                            )
                    for B in range(BPC):
                        # unnormalized eviction into opT; s = 4u + sp
                        for sg in range(2):
                            nc.vector.tensor_copy(
                                opT[sg * DK : (sg + 1) * DK,
                                    2 * u : 2 * u + 2, B, :],
                                pv[B][0:DK, :].rearrange(
                                    "d (sp t) -> d sp t", t=TPB
                                )[:, sg::2, :],
                            )
                        r_ = B * NQC + u
                        nc.vector.tensor_copy(
                            sums[r_ // 4][32 * (r_ % 4) : 32 * (r_ % 4) + 1, :],
                            pv[B][DK : DK + 1, :],
                        )

                rec = [
                    nrm_pool.tile([P, QC], mybir.dt.float32r, tag="rec",
                                  name=f"rec{_i}")
                    for _i in range(2)
                ]
                with nc.allow_low_precision(reason="softmax denominators"):
                    for _i in range(2):
                        nc.vector.reciprocal(rec[_i][:], sums[_i][:])
                for B in range(BPC):
                    for u in range(NQC):
                        feed.append(make_normalize_piece(opT, rec, B, u))
                for B in range(BPC):
                    for half in range(2):
                        feed.append(make_outproj_piece(opT, n, B, half))
                if next_state is not None:
                    tiles = next_state[0]

            while feed:
                feed.pop(0)()

    nc.compile()
    return nc


_CACHED_NC = None


def get_nc():
    global _CACHED_NC
    if _CACHED_NC is None:
        _CACHED_NC = build_nc()
    return _CACHED_NC


def make_in_maps(inputs):
    x = np.ascontiguousarray(np.asarray(inputs["x"], dtype=np.float32))
    Wq = np.asarray(inputs["Wq"], dtype=np.float32)
    Wk = np.asarray(inputs["Wk"], dtype=np.float32)
    Wv = np.asarray(inputs["Wv"], dtype=np.float32)
    Wo = np.asarray(inputs["Wo"], dtype=np.float32)

    def cast(a):
        return np.ascontiguousarray(a).astype(ml_dtypes.bfloat16)

    wqT = cast(Wq.T)
    wkT = cast(Wk.T)
    wvT = cast(Wv.T)
    woT = cast(Wo.T)
    xr = x.reshape(N, L, E)

    in_maps = []
    for c in range(NC):
        xc = np.concatenate(
            [xr[n, 256 * c : 256 * (c + 1), :] for n in range(N)], axis=0
        )
        in_maps.append(
            {
                "xTc": cast(xc.T),
                "wqT": wqT,
                "wkT": wkT,
                "wvT": wvT,
                "woT": woT,
            }
        )
    return in_maps


def kernel(x, Wq, Wk, Wv, Wo):
    in_maps = make_in_maps({"x": x, "Wq": Wq, "Wk": Wk, "Wv": Wv, "Wo": Wo})
    res = run_bass_kernel_spmd(get_nc(), in_maps, list(range(NC)))
    out = np.empty((N, L, E), dtype=np.float32)
    for c in range(NC):
        o = res.results[c]["outp"].reshape(N, TPN, E)
        out[:, 256 * c : 256 * (c + 1), :] = o
    return out



# revision 3
# speedup vs baseline: 1.0501x; 1.0501x over previous
"""Multi-head attention (N=4, L=2048, E=1024, H=16, DK=64) on 8 TRN2 cores.

The reference splits heads with a PLAIN RESHAPE (n, l, H*DK) -> (n, H, l, DK),
so "head" h is really a contiguous block of 128 tokens, and the 2048 attention
positions inside it are (token, s) pairs where s indexes sixteen 64-wide
E-slices.  Per (batch, block):
    Qb = q[n, 128b:128b+128, :].reshape(2048, 64)   (same for K, V)
    out_block = softmax(Qb Kb^T / 8) Vb  -> reshape(128, E) -> rows of out
Positions are processed in permuted order p' = 128*s + tok (a permutation of
the softmax axis; unpermuted on the way out).

Sharding: core c owns token rows [n, 256c : 256c+256) for every batch n (two
128-token blocks per batch).  Outputs are disjoint rows; the host scatters.
Each core gets the full weights (bf16, all resident in SBUF) and only its own
x columns.

v2 changes vs baseline:
  - DMA order: x batch0 first, then wk/wq/wv in 128-col chunks so the first
    projection chains start ~2us in instead of ~30us.
  - Q/K projection psums cover an a2-PAIR [128, 2, 256] (one PSUM bank);
    evictions become 4 copies of [64, 2, 128] (half the DVE instructions).
  - Part of the exp work moves from ScalarE to the (otherwise idle) Pool
    engine as pow(e^(1/8), scores) for j in POOL_JS.
  - Softmax denominator reciprocal via reciprocal_approx_fast (5x faster).
"""

import ml_dtypes
import numpy as np

import concourse.bass as bass
import concourse.mybir as mybir
import concourse.tile as tile
from concourse import bacc
from concourse.bass_utils import run_bass_kernel_spmd

N, L, E, H = 4, 2048, 1024, 16
DK = E // H  # 64
NC = 8
BPC = 2  # token blocks per core per batch
TPB = 128  # tokens per block
TPN = BPC * TPB  # 256 tokens per batch per core
TC = N * TPN  # 1024 tokens per core
P = 128
QC = 512  # q' chunk
NQC = 2048 // QC  # 4
NKT = 2048 // P  # 16 key tiles (= s values)
ET = E // P  # 8

F32 = mybir.dt.float32
BF16 = mybir.dt.bfloat16
MM_DT = BF16

# Pool-engine pow measured ~85us per [128,512] on HW (software Q7
# transcendental) and DVE pow fails to compile -- exp stays on ScalarE.
POOL_JS = frozenset()


def build_nc(pool_js=POOL_JS):
    nc = bacc.Bacc("TRN2", target_bir_lowering=False, debug=False, num_devices=NC)

    xTc = nc.dram_tensor("xTc", [E, TC], MM_DT, kind="ExternalInput").ap()
    wqT = nc.dram_tensor("wqT", [E, E], MM_DT, kind="ExternalInput").ap()
    wkT = nc.dram_tensor("wkT", [E, E], MM_DT, kind="ExternalInput").ap()
    wvT = nc.dram_tensor("wvT", [E, E], MM_DT, kind="ExternalInput").ap()
    woT = nc.dram_tensor("woT", [E, E], MM_DT, kind="ExternalInput").ap()
    outp = nc.dram_tensor("outp", [TC, E], F32, kind="ExternalOutput").ap()

    with tile.TileContext(nc) as tc:
        with (
            tc.tile_pool(name="const", bufs=1) as const,
            tc.tile_pool(name="wpool", bufs=1) as wpool,
            tc.tile_pool(name="xv", bufs=2) as xv_pool,
            tc.tile_pool(name="qk1", bufs=2) as qk1_pool,
            tc.tile_pool(name="expp", bufs=6) as exp_pool,
            tc.tile_pool(name="opt", bufs=2) as opt_pool,
            tc.tile_pool(name="nrm", bufs=2) as nrm_pool,
            tc.tile_pool(name="ops", bufs=2) as op_pool,
            tc.tile_pool(name="scps", bufs=2, space="PSUM") as sc_psum,
            tc.tile_pool(name="pvps", bufs=4, space="PSUM") as pv_psum,
        ):
            ones_f32 = const.tile([P, P], F32)
            nc.vector.memset(ones_f32[:], 1.0)
            ones_r = const.tile([P, P], mybir.dt.float32r)
            nc.vector.tensor_copy(ones_r[:], ones_f32[:])
            cexp = None
            if pool_js:
                cexp = const.tile([P, BPC, QC], F32)
                nc.vector.memset(cexp[:], float(np.exp(1.0 / np.sqrt(DK))))

            # ---- resident x (batch 0 first; weights interleave below) ----
            x_sb = xv_pool.tile([P, ET, TC], MM_DT, tag="xv", name="x_sb")
            xr_ = xTc.rearrange("(a p) t -> p a t", p=P)
            nc.sync.dma_start(out=x_sb[:, :, 0:TPN], in_=xr_[:, :, 0:TPN])

            def load_w(w_dram, nm, chunked=True):
                w_sb = wpool.tile([P, ET, E], MM_DT, tag=nm, name=nm)
                wr = w_dram.rearrange("(a p) d -> p a d", p=P)
                if chunked:
                    for a2 in range(ET):
                        nc.sync.dma_start(
                            out=w_sb[:, :, a2 * P : (a2 + 1) * P],
                            in_=wr[:, :, a2 * P : (a2 + 1) * P],
                        )
                else:
                    nc.sync.dma_start(out=w_sb[:], in_=wr)
                return w_sb

            wk_sb = load_w(wkT, "wk")
            wq_sb = load_w(wqT, "wq")
            wv_sb = load_w(wvT, "wv")
            for n_ in range(1, N):
                nc.sync.dma_start(
                    out=x_sb[:, :, n_ * TPN : (n_ + 1) * TPN],
                    in_=xr_[:, :, n_ * TPN : (n_ + 1) * TPN],
                )
            wo_sb = load_w(woT, "wo", chunked=False)

            def project_batch(n):
                """Returns (tiles, [chunk emitters]) so projection work for
                batch n can be interleaved into batch n-1's attention units."""
                v_sb = qk1_pool.tile(
                    [P, BPC, NKT, DK + 1], MM_DT, tag="v", name="v_sb"
                )
                q1t = qk1_pool.tile([P, 2048], MM_DT, tag="q1", name="q1t")
                k1t = qk1_pool.tile([P, 2048], MM_DT, tag="k1", name="k1t")
                groups = []

                # K and Q over an a2-pair: psum [128, 2, 256] (one bank),
                # evicted as 4 strided copies [64, 2, 128] into the permuted
                # q1t/k1t layout (s = 4p + 2*sub + rh).
                def qk_group(w_sb, dst, pr_):
                    def emit():
                        ps = pv_psum.tile([P, 2, TPN], F32, tag="pv", name="qkps")
                        for sub in range(2):
                            a2 = 2 * pr_ + sub
                            for a in range(ET):
                                nc.tensor.matmul(
                                    ps[:, sub, :],
                                    w_sb[:, a, a2 * P : (a2 + 1) * P],
                                    x_sb[:, a, n * TPN : (n + 1) * TPN],
                                    start=(a == 0),
                                    stop=(a == ET - 1),
                                )
                        psr = ps.rearrange("q s (b t) -> q s b t", t=TPB)
                        dr = dst.rearrange("q (s t) -> q s t", t=TPB)
                        for rh in range(2):
                            for B in range(BPC):
                                s0 = 4 * pr_ + rh
                                nc.vector.tensor_copy(
                                    dr[B * DK : (B + 1) * DK, s0 : s0 + 3 : 2, :],
                                    psr[rh * DK : (rh + 1) * DK, :, B, :],
                                )
                    return emit

                def v_group(B, eh):
                    def emit():
                        tok0 = n * TPN + B * TPB
                        ps = pv_psum.tile([P, 512], F32, tag="pv", name="vps")
                        for a in range(ET):
                            nc.tensor.matmul(
                                ps[:],
                                x_sb[:, a, tok0 : tok0 + TPB],
                                wv_sb[:, a, eh * 512 : (eh + 1) * 512],
                                start=(a == 0),
                                stop=(a == ET - 1),
                            )
                        nc.vector.tensor_copy(
                            v_sb[:, B, eh * 8 : (eh + 1) * 8, 0:DK],
                            ps.rearrange("p (s d) -> p s d", d=DK),
                        )
                    return emit

                def ones_group():
                    nc.vector.tensor_copy(
                        v_sb[:, :, :, DK], ones_f32[:, 0 : BPC * NKT]
                    )

                for pr_ in range(ET // 2):
                    groups.append(qk_group(wk_sb, k1t, pr_))
                for pr_ in range(ET // 2):
                    groups.append(qk_group(wq_sb, q1t, pr_))
                for B in range(BPC):
                    for eh in range(2):
                        groups.append(v_group(B, eh))
                groups.append(ones_group)
                return (v_sb, q1t, k1t), groups

            # ---- per batch: project, attend, normalize, out-project ----
            # `feed` holds deferred fine-grained work (next batch's projection
            # chunks, previous batch's normalize/out-proj pieces) drained one
            # item per key-tile so the PE stream never starves ScalarE.
            tiles, groups = project_batch(0)
            for g in groups:
                g()
            next_state = None
            feed = []

            def make_normalize_piece(opT, rec, B, u):
                def emit():
                    r_ = B * NQC + u
                    rp = 32 * (r_ % 4)
                    bcp = pv_psum.tile([P, QC], F32, tag="pv", name="bcp")
                    nc.tensor.matmul(
                        bcp[:],
                        ones_r[rp : rp + 1, :],
                        rec[r_ // 4][rp : rp + 1, :],
                        start=True,
                        stop=True,
                        tile_position=(rp, 0),
                    )
                    for sg in range(2):
                        tgt = opT[sg * DK : (sg + 1) * DK,
                                  2 * u : 2 * u + 2, B, :]
                        nc.vector.tensor_mul(
                            tgt,
                            tgt,
                            bcp[sg * DK : (sg + 1) * DK, :].rearrange(
                                "d (sp t) -> d sp t", t=TPB
                            )[:, sg::2, :],
                        )
                return emit

            def make_outproj_piece(opT, n, B, half):
                def emit():
                    ps = pv_psum.tile([P, 512], F32, tag="pv", name="opps")
                    for a2 in range(ET):
                        nc.tensor.matmul(
                            ps[:],
                            opT[:, a2, B, :],
                            wo_sb[:, a2, half * 512 : (half + 1) * 512],
                            start=(a2 == 0),
                            stop=(a2 == ET - 1),
                        )
                    op_sb = op_pool.tile([P, 512], F32, tag="op")
                    nc.vector.tensor_copy(op_sb[:], ps[:])
                    r0 = n * TPN + B * TPB
                    nc.sync.dma_start(
                        out=outp[r0 : r0 + TPB, half * 512 : (half + 1) * 512],
                        in_=op_sb[:],
                    )
                return emit

            for n in range(N):
                while feed:
                    feed.pop(0)()  # safety drain before slot-reusing allocs
                v_sb, q1t, k1t = tiles
                if n + 1 < N:
                    next_state = project_batch(n + 1)
                    feed.extend(next_state[1])
                opT = opt_pool.tile([P, ET, BPC, TPB], MM_DT, tag="opT", name="opT")
                # denominator rows live at 32-aligned partitions of two tiles
                sums = [
                    nrm_pool.tile([P, QC], F32, tag="sums", name=f"sums{_i}")
                    for _i in range(2)
                ]
                for u in range(NQC):
                    # drain deferred work in small lumps at unit boundaries
                    take = (len(feed) + NQC - 1 - u) // (NQC - u) if feed else 0
                    for _ in range(min(take, len(feed))):
                        feed.pop(0)()
                    qsl = slice(u * QC, (u + 1) * QC)
                    pv = [
                        pv_psum.tile([DK + 1, QC], F32, tag="pv", name=f"pv{_b}")
                        for _b in range(BPC)
                    ]
                    for j in range(NKT):
                        sc = sc_psum.tile([P, BPC, QC], F32, tag="sc")
                        ksl = slice(j * TPB, (j + 1) * TPB)
                        for B in range(BPC):
                            bsl = slice(B * DK, (B + 1) * DK)
                            nc.tensor.matmul(
                                sc[:, B, :],
                                k1t[bsl, ksl],
                                q1t[bsl, qsl],
                                start=True,
                                stop=True,
                            )
                        exps = exp_pool.tile([P, BPC, QC], MM_DT, tag="exps")
                        if j in pool_js:
                            nc.gpsimd.tensor_tensor(
                                exps[:], cexp[:], sc[:], mybir.AluOpType.pow
                            )
                        else:
                            nc.scalar.activation(
                                exps[:],
                                sc[:],
                                mybir.ActivationFunctionType.Exp,
                                scale=1.0 / np.sqrt(DK),
                            )
                        for B in range(BPC):
                            nc.tensor.matmul(
                                pv[B][:],
                                v_sb[:, B, j, :],
                                exps[:, B, :],
                                start=(j == 0),
                                stop=(j == NKT - 1),
                            )
                    for B in range(BPC):
                        # unnormalized eviction into opT; s = 4u + sp
                        for sg in range(2):
                            nc.vector.tensor_copy(
                                opT[sg * DK : (sg + 1) * DK,
                                    2 * u : 2 * u + 2, B, :],
                                pv[B][0:DK, :].rearrange(
                                    "d (sp t) -> d sp t", t=TPB
                                )[:, sg::2, :],
                            )
                        r_ = B * NQC + u
                        nc.vector.tensor_copy(
                            sums[r_ // 4][32 * (r_ % 4) : 32 * (r_ % 4) + 1, :],
                            pv[B][DK : DK + 1, :],
                        )

                rec = [
                    nrm_pool.tile([P, QC], mybir.dt.float32r, tag="rec",
                                  name=f"rec{_i}")
                    for _i in range(2)
                ]
                recf = [
                    nrm_pool.tile([P, QC], F32, tag="recf", name=f"recf{_i}")
                    for _i in range(2)
                ]
                with nc.allow_low_precision(reason="softmax denominators"):
                    for _i in range(2):
                        nc.vector.reciprocal_approx_fast(
                            out=recf[_i][:], in_=sums[_i][:]
                        )
                        nc.vector.tensor_copy(rec[_i][:], recf[_i][:])
                for B in range(BPC):
                    for u in range(NQC):
                        feed.append(make_normalize_piece(opT, rec, B, u))
                for B in range(BPC):
                    for half in range(2):
                        feed.append(make_outproj_piece(opT, n, B, half))
                if next_state is not None:
                    tiles = next_state[0]

            while feed:
                feed.pop(0)()

    nc.compile()
    return nc


_CACHED_NC = None


def get_nc():
    global _CACHED_NC
    if _CACHED_NC is None:
        _CACHED_NC = build_nc()
    return _CACHED_NC


def make_in_maps(inputs):
    x = np.ascontiguousarray(np.asarray(inputs["x"], dtype=np.float32))
    Wq = np.asarray(inputs["Wq"], dtype=np.float32)
    Wk = np.asarray(inputs["Wk"], dtype=np.float32)
    Wv = np.asarray(inputs["Wv"], dtype=np.float32)
    Wo = np.asarray(inputs["Wo"], dtype=np.float32)

    def cast(a):
        return np.ascontiguousarray(a).astype(ml_dtypes.bfloat16)

    wqT = cast(Wq.T)
    wkT = cast(Wk.T)
    wvT = cast(Wv.T)
    woT = cast(Wo.T)
    xr = x.reshape(N, L, E)

    in_maps = []
    for c in range(NC):
        xc = np.concatenate(
            [xr[n, 256 * c : 256 * (c + 1), :] for n in range(N)], axis=0
        )
        in_maps.append(
            {
                "xTc": cast(xc.T),
                "wqT": wqT,
                "wkT": wkT,
                "wvT": wvT,
                "woT": woT,
            }
        )
    return in_maps


def kernel(x, Wq, Wk, Wv, Wo):
    in_maps = make_in_maps({"x": x, "Wq": Wq, "Wk": Wk, "Wv": Wv, "Wo": Wo})
    res = run_bass_kernel_spmd(get_nc(), in_maps, list(range(NC)))
    out = np.empty((N, L, E), dtype=np.float32)
    for c in range(NC):
        o = res.results[c]["outp"].reshape(N, TPN, E)
        out[:, 256 * c : 256 * (c + 1), :] = o
    return out


# revision 11
# speedup vs baseline: 1.0728x; 1.0217x over previous
"""Multi-head attention (N=4, L=2048, E=1024, H=16, DK=64) on 8 TRN2 cores.

The reference splits heads with a PLAIN RESHAPE (n, l, H*DK) -> (n, H, l, DK),
so "head" h is really a contiguous block of 128 tokens, and the 2048 attention
positions inside it are (token, s) pairs where s indexes sixteen 64-wide
E-slices.  Per (batch, block):
    Qb = q[n, 128b:128b+128, :].reshape(2048, 64)   (same for K, V)
    out_block = softmax(Qb Kb^T / 8) Vb  -> reshape(128, E) -> rows of out
Positions are processed in permuted order p' = 128*s + tok (a permutation of
the softmax axis; unpermuted on the way out).

Sharding: core c owns token rows [n, 256c : 256c+256) for every batch n (two
128-token blocks per batch).  Outputs are disjoint rows; the host scatters.
Each core gets the full weights (bf16, all resident in SBUF) and only its own
x columns.

v3: the attention inner loop is software-pipelined at the instruction level.
PE emission order per key tile j is [scores(j); one deferred-work piece;
PV(j-1)], so the ~1.3us exp latency (ScalarE) of tile j is hidden behind
scores(j+1) plus a feed piece, and the ScalarE exp stream (the throughput
floor, ~1.15us per key tile) never starves.  All projection / normalize /
out-projection work is chopped into ~0.5-1.7us pieces drained one per j from
`feed`.  PSUM: scores 2x2 banks, PV accumulators 2x1, feed chains 2x1.
"""

import ml_dtypes
import numpy as np

import concourse.bass as bass
import concourse.mybir as mybir
import concourse.tile as tile
from concourse import bacc
from concourse.bass_utils import run_bass_kernel_spmd

N, L, E, H = 4, 2048, 1024, 16
DK = E // H  # 64
NC = 8
BPC = 2  # token blocks per core per batch
TPB = 128  # tokens per block
TPN = BPC * TPB  # 256 tokens per batch per core
TC = N * TPN  # 1024 tokens per core
P = 128
QC = 512  # q' chunk
NQC = 2048 // QC  # 4
NKT = 2048 // P  # 16 key tiles (= s values)
ET = E // P  # 8

F32 = mybir.dt.float32
BF16 = mybir.dt.bfloat16
MM_DT = BF16


def build_nc():
    nc = bacc.Bacc("TRN2", target_bir_lowering=False, debug=False, num_devices=NC)

    xTc = nc.dram_tensor("xTc", [E, TC], MM_DT, kind="ExternalInput").ap()
    wqT = nc.dram_tensor("wqT", [E, E], MM_DT, kind="ExternalInput").ap()
    wkT = nc.dram_tensor("wkT", [E, E], MM_DT, kind="ExternalInput").ap()
    wvT = nc.dram_tensor("wvT", [E, E], MM_DT, kind="ExternalInput").ap()
    woT = nc.dram_tensor("woT", [E, E], MM_DT, kind="ExternalInput").ap()
    outp = nc.dram_tensor("outp", [TC, E], F32, kind="ExternalOutput").ap()

    with tile.TileContext(nc) as tc:
        with (
            tc.tile_pool(name="const", bufs=1) as const,
            tc.tile_pool(name="wpool", bufs=1) as wpool,
            tc.tile_pool(name="xv", bufs=2) as xv_pool,
            tc.tile_pool(name="qk1", bufs=2) as qk1_pool,
            tc.tile_pool(name="expp", bufs=6) as exp_pool,
            tc.tile_pool(name="opt", bufs=2) as opt_pool,
            tc.tile_pool(name="nrm", bufs=2) as nrm_pool,
            tc.tile_pool(name="ops", bufs=2) as op_pool,
            tc.tile_pool(name="scps", bufs=2, space="PSUM") as sc_psum,
            tc.tile_pool(name="pvps", bufs=2, space="PSUM") as pv_psum,
            tc.tile_pool(name="fdps", bufs=2, space="PSUM") as fd_psum,
        ):
            ones_f32 = const.tile([P, P], F32)
            nc.vector.memset(ones_f32[:], 1.0)
            ones_r = const.tile([P, P], mybir.dt.float32r)
            nc.vector.tensor_copy(ones_r[:], ones_f32[:])

            # ---- resident x (batch 0 first; weights interleave below) ----
            x_sb = xv_pool.tile([P, ET, TC], MM_DT, tag="xv", name="x_sb")
            xr_ = xTc.rearrange("(a p) t -> p a t", p=P)
            nc.sync.dma_start(out=x_sb[:, :, 0:TPN], in_=xr_[:, :, 0:TPN])

            def load_w(w_dram, nm, chunked=True):
                w_sb = wpool.tile([P, ET, E], MM_DT, tag=nm, name=nm)
                wr = w_dram.rearrange("(a p) d -> p a d", p=P)
                if chunked:
                    for a2 in range(ET):
                        nc.sync.dma_start(
                            out=w_sb[:, :, a2 * P : (a2 + 1) * P],
                            in_=wr[:, :, a2 * P : (a2 + 1) * P],
                        )
                else:
                    nc.sync.dma_start(out=w_sb[:], in_=wr)
                return w_sb

            wk_sb = load_w(wkT, "wk")
            wq_sb = load_w(wqT, "wq")
            wv_sb = load_w(wvT, "wv")
            for n_ in range(1, N):
                nc.sync.dma_start(
                    out=x_sb[:, :, n_ * TPN : (n_ + 1) * TPN],
                    in_=xr_[:, :, n_ * TPN : (n_ + 1) * TPN],
                )
            wo_sb = load_w(woT, "wo", chunked=False)

            def project_batch(n):
                """Returns ((v_sb, q1t, k1t), pieces) with each piece ~one
                PSUM-chain segment so the feed can drain one piece per key
                tile.  Piece order: critical-first for same-batch startup."""
                v_sb = qk1_pool.tile(
                    [P, BPC, NKT, DK + 1], MM_DT, tag="v", name="v_sb"
                )
                q1t = qk1_pool.tile([P, 2048], MM_DT, tag="q1", name="q1t")
                k1t = qk1_pool.tile([P, 2048], MM_DT, tag="k1", name="k1t")

                def qk_pieces(w_sb, dst, pr_):
                    cell = {}

                    def mk_mm(sub):
                        def em():
                            if sub == 0:
                                cell["ps"] = fd_psum.tile(
                                    [P, 2, TPN], F32, tag="fd", name="qkps"
                                )
                            ps = cell["ps"]
                            a2 = 2 * pr_ + sub
                            for a in range(ET):
                                nc.tensor.matmul(
                                    ps[:, sub, :],
                                    w_sb[:, a, a2 * P : (a2 + 1) * P],
                                    x_sb[:, a, n * TPN : (n + 1) * TPN],
                                    start=(a == 0),
                                    stop=(a == ET - 1),
                                )
                        return em

                    def evict():
                        ps = cell["ps"]
                        psr = ps.rearrange("q s (b t) -> q s b t", t=TPB)
                        dr = dst.rearrange("q (s t) -> q s t", t=TPB)
                        for rh in range(2):
                            for B in range(BPC):
                                s0 = 4 * pr_ + rh
                                nc.vector.tensor_copy(
                                    dr[B * DK : (B + 1) * DK, s0 : s0 + 3 : 2, :],
                                    psr[rh * DK : (rh + 1) * DK, :, B, :],
                                )

                    return [mk_mm(0), mk_mm(1), evict]

                def v_pieces(B, eh):
                    cell = {}
                    tok0 = n * TPN + B * TPB

                    def mk_mm(half):
                        def em():
                            if half == 0:
                                cell["ps"] = fd_psum.tile(
                                    [P, 512], F32, tag="fd", name="vps"
                                )
                            ps = cell["ps"]
                            for a in range(4 * half, 4 * half + 4):
                                nc.tensor.matmul(
                                    ps[:],
                                    x_sb[:, a, tok0 : tok0 + TPB],
                                    wv_sb[:, a, eh * 512 : (eh + 1) * 512],
                                    start=(a == 0),
                                    stop=(a == ET - 1),
                                )
                        return em

                    def evict():
                        nc.vector.tensor_copy(
                            v_sb[:, B, eh * 8 : (eh + 1) * 8, 0:DK],
                            cell["ps"].rearrange("p (s d) -> p s d", d=DK),
                        )

                    return [mk_mm(0), mk_mm(1), evict]

                def ones_piece():
                    nc.vector.tensor_copy(
                        v_sb[:, :, :, DK], ones_f32[:, 0 : BPC * NKT]
                    )

                # (deadline_offset, piece): the piece MUST be emitted by the
                # drain at batch_base+offset.  scores(j) consume k-pair j//4
                # and q-pair u and are emitted BEFORE the drain of their slot
                # (hence -2 margins); PV(j-1) consumes v/ones and is emitted
                # after the drain.
                pieces = (
                    [(-2, p) for p in qk_pieces(wk_sb, k1t, 0)]
                    + [(-2, p) for p in qk_pieces(wq_sb, q1t, 0)]
                    + [(0, p) for p in v_pieces(0, 0)]
                    + [(0, p) for p in v_pieces(1, 0)]
                    + [(0, ones_piece)]
                    + [(2, p) for p in qk_pieces(wk_sb, k1t, 1)]
                    + [(6, p) for p in qk_pieces(wk_sb, k1t, 2)]
                    + [(7, p) for p in v_pieces(0, 1)]
                    + [(7, p) for p in v_pieces(1, 1)]
                    + [(10, p) for p in qk_pieces(wk_sb, k1t, 3)]
                    + [(14, p) for p in qk_pieces(wq_sb, q1t, 1)]
                    + [(31, p) for p in qk_pieces(wq_sb, q1t, 2)]
                    + [(48, p) for p in qk_pieces(wq_sb, q1t, 3)]
                )
                return (v_sb, q1t, k1t), pieces

            def make_normalize_piece(opT, rec, B, u):
                def emit():
                    rp = 32 * (2 * (u % 2) + B)
                    bcp = fd_psum.tile([P, QC], F32, tag="fd", name="bcp")
                    nc.tensor.matmul(
                        bcp[:],
                        ones_r[rp : rp + 1, :],
                        rec[u // 2][rp : rp + 1, :],
                        start=True,
                        stop=True,
                        tile_position=(rp, 0),
                    )
                    for sg in range(2):
                        tgt = opT[sg * DK : (sg + 1) * DK,
                                  2 * u : 2 * u + 2, B, :]
                        nc.vector.tensor_mul(
                            tgt,
                            tgt,
                            bcp[sg * DK : (sg + 1) * DK, :].rearrange(
                                "d (sp t) -> d sp t", t=TPB
                            )[:, sg::2, :],
                        )
                return emit

            def make_outproj_pieces(opT, n, B, half):
                cell = {}

                def chain():
                    cell["ps"] = fd_psum.tile([P, 512], F32, tag="fd",
                                              name="opps")
                    ps = cell["ps"]
                    for a2 in range(ET):
                        nc.tensor.matmul(
                            ps[:],
                            opT[:, a2, B, :],
                            wo_sb[:, a2, half * 512 : (half + 1) * 512],
                            start=(a2 == 0),
                            stop=(a2 == ET - 1),
                        )

                def evict():
                    op_sb = op_pool.tile([P, 512], F32, tag="op")
                    nc.vector.tensor_copy(op_sb[:], cell["ps"][:])
                    r0 = n * TPN + B * TPB
                    nc.sync.dma_start(
                        out=outp[r0 : r0 + TPB, half * 512 : (half + 1) * 512],
                        in_=op_sb[:],
                    )

                return [chain, evict]

            # ---- pipelined batches ----
            # feed: deadline-sorted (key, seq, piece).  Forced pops at each
            # drain guarantee producers are emitted before their consumers;
            # pacing spreads the rest one piece per key tile.
            import bisect

            SPB = NQC * (NKT + 1)  # drain slots per batch
            feed = []
            seq_counter = [0]

            def push(key, piece):
                item = (key, seq_counter[0], piece)
                seq_counter[0] += 1
                bisect.insort(feed, item)

            def drain(slot, slots_left):
                while feed and feed[0][0] <= slot:
                    feed.pop(0)[2]()
                if feed:
                    want = (len(feed) + slots_left - 1) // max(slots_left, 1)
                    for _ in range(min(max(want, 1), 2, len(feed))):
                        feed.pop(0)[2]()

            tiles, pieces0 = project_batch(0)
            for off, p in pieces0:
                if off <= 0:
                    p()  # batch 0's critical prefix runs eagerly
                else:
                    push(off, p)
            next_state = None

            for n in range(N):
                base = n * SPB
                v_sb, q1t, k1t = tiles
                if n + 1 < N:
                    next_state = project_batch(n + 1)
                    for off, p in next_state[1]:
                        push((n + 1) * SPB + off, p)
                opT = opt_pool.tile([P, ET, BPC, TPB], MM_DT, tag="opT",
                                    name="opT")
                # denominators: tile u//2, partition 32*(2*(u%2)+B)
                sums = [
                    nrm_pool.tile([P, QC], F32, tag="sums", name=f"sums{_i}")
                    for _i in range(2)
                ]
                # only 4 partitions carry denominators; initialize the rest
                # so the full-tile reciprocal reads defined data
                for _i in range(2):
                    nc.vector.memset(sums[_i][:], 1.0)
                rec = [
                    nrm_pool.tile([P, QC], mybir.dt.float32r, tag="rec",
                                  name=f"rec{_i}")
                    for _i in range(2)
                ]
                recf = nrm_pool.tile([P, QC], F32, tag="recf", name="recf")

                for u in range(NQC):
                    qsl = slice(u * QC, (u + 1) * QC)
                    pv = [
                        pv_psum.tile([DK + 1, QC], F32, tag="pv",
                                     name=f"pv{_b}")
                        for _b in range(BPC)
                    ]
                    exps_ring = []
                    for j in range(NKT + 1):
                        if j < NKT:
                            sc = sc_psum.tile([P, BPC, QC], F32, tag="sc")
                            ksl = slice(j * TPB, (j + 1) * TPB)
                            for B in range(BPC):
                                bsl = slice(B * DK, (B + 1) * DK)
                                nc.tensor.matmul(
                                    sc[:, B, :],
                                    k1t[bsl, ksl],
                                    q1t[bsl, qsl],
                                    start=True,
                                    stop=True,
                                )
                            exps = exp_pool.tile([P, BPC, QC], MM_DT,
                                                 tag="exps")
                            nc.scalar.activation(
                                exps[:],
                                sc[:],
                                mybir.ActivationFunctionType.Exp,
                                scale=1.0 / np.sqrt(DK),
                            )
                            exps_ring.append(exps)
                        # deferred work between scores(j) and PV(j-1) hides
                        # the exp latency of tile j-1
                        drain(base + u * (NKT + 1) + j,
                              (NQC - u) * (NKT + 1) - j)
                        if j >= 1:
                            jj = j - 1
                            for B in range(BPC):
                                nc.tensor.matmul(
                                    pv[B][:],
                                    v_sb[:, B, jj, :],
                                    exps_ring[jj][:, B, :],
                                    start=(jj == 0),
                                    stop=(jj == NKT - 1),
                                )
                    for B in range(BPC):
                        # unnormalized eviction into opT; s = 4u + sp
                        for sg in range(2):
                            nc.vector.tensor_copy(
                                opT[sg * DK : (sg + 1) * DK,
                                    2 * u : 2 * u + 2, B, :],
                                pv[B][0:DK, :].rearrange(
                                    "d (sp t) -> d sp t", t=TPB
                                )[:, sg::2, :],
                            )
                        rp = 32 * (2 * (u % 2) + B)
                        nc.vector.tensor_copy(
                            sums[u // 2][rp : rp + 1, :],
                            pv[B][DK : DK + 1, :],
                        )
                    if u % 2 == 1:
                        # this sums tile is complete: reciprocal now, then
                        # queue the normalize pieces for its two units
                        half = u // 2
                        with nc.allow_low_precision(
                            reason="softmax denominators"
                        ):
                            nc.vector.reciprocal_approx_fast(
                                out=recf[:], in_=sums[half][:]
                            )
                            nc.vector.tensor_copy(rec[half][:], recf[:])
                        # normalize u01 must finish before this batch ends
                        # (rec[0] ring reuse); u23 early next batch
                        nkey = base + 40 if u == 1 else base + SPB + 8
                        for uu in (u - 1, u):
                            for B in range(BPC):
                                push(nkey,
                                     make_normalize_piece(opT, rec, B, uu))
                for B in range(BPC):
                    for half in range(2):
                        for p in make_outproj_pieces(opT, n, B, half):
                            push(base + SPB + 14, p)
                if next_state is not None:
                    tiles = next_state[0]

            while feed:
                feed.pop(0)[2]()

    nc.compile()
    return nc


_CACHED_NC = None


def get_nc():
    global _CACHED_NC
    if _CACHED_NC is None:
        _CACHED_NC = build_nc()
    return _CACHED_NC


def make_in_maps(inputs):
    x = np.ascontiguousarray(np.asarray(inputs["x"], dtype=np.float32))
    Wq = np.asarray(inputs["Wq"], dtype=np.float32)
    Wk = np.asarray(inputs["Wk"], dtype=np.float32)
    Wv = np.asarray(inputs["Wv"], dtype=np.float32)
    Wo = np.asarray(inputs["Wo"], dtype=np.float32)

    def cast(a):
        return np.ascontiguousarray(a).astype(ml_dtypes.bfloat16)

    wqT = cast(Wq.T)
    wkT = cast(Wk.T)
    wvT = cast(Wv.T)
    woT = cast(Wo.T)
    xr = x.reshape(N, L, E)

    in_maps = []
    for c in range(NC):
        xc = np.concatenate(
            [xr[n, 256 * c : 256 * (c + 1), :] for n in range(N)], axis=0
        )
        in_maps.append(
            {
                "xTc": cast(xc.T),
                "wqT": wqT,
                "wkT": wkT,
                "wvT": wvT,
                "woT": woT,
            }
        )
    return in_maps


def kernel(x, Wq, Wk, Wv, Wo):
    in_maps = make_in_maps({"x": x, "Wq": Wq, "Wk": Wk, "Wv": Wv, "Wo": Wo})
    res = run_bass_kernel_spmd(get_nc(), in_maps, list(range(NC)))
    out = np.empty((N, L, E), dtype=np.float32)
    for c in range(NC):
        o = res.results[c]["outp"].reshape(N, TPN, E)
        out[:, 256 * c : 256 * (c + 1), :] = o
    return out


# revision 13
# speedup vs baseline: 1.1543x; 1.0759x over previous
"""Multi-head attention (N=4, L=2048, E=1024, H=16, DK=64) on 8 TRN2 cores.

The reference splits heads with a PLAIN RESHAPE (n, l, H*DK) -> (n, H, l, DK),
so "head" h is really a contiguous block of 128 tokens, and the 2048 attention
positions inside it are (token, s) pairs where s indexes sixteen 64-wide
E-slices.  Per (batch, block):
    Qb = q[n, 128b:128b+128, :].reshape(2048, 64)   (same for K, V)
    out_block = softmax(Qb Kb^T / 8) Vb  -> reshape(128, E) -> rows of out
Positions are processed in permuted order p' = 128*s + tok (a permutation of
the softmax axis; unpermuted on the way out).

Sharding: core c owns token rows [n, 256c : 256c+256) for every batch n (two
128-token blocks per batch).  Outputs are disjoint rows; the host scatters.
Each core gets the full weights (bf16, all resident in SBUF) and only its own
x columns.

v4 pipeline: the ScalarE exp stream (~1.15us per key tile, 256 tiles) is the
throughput floor; everything else is scheduled around keeping it saturated.
  - PE emission per key tile j: [scores(j); ~430ns of deferred work; PV(j-1)]
    so the exp latency of tile j-1 is hidden and per-tile PE time stays just
    under the exp time.
  - All projection / normalize / out-projection work is chopped into
    cost-classified pieces (heavy ~430ns PE, light DVE-only) in a
    deadline-keyed queue; forced pops guarantee producers are emitted before
    consumers, budget-based pacing keeps the PE load smooth.
  - DMA initiation order: x(batch0), Wk/Wq first columns, Wv first half --
    the minimal set for the first attention unit -- then the rest.
  - Softmax denominators: per-unit half-tile reciprocal_approx_fast so
    normalize/out-projection of the last unit is the only tail work.
"""

import bisect

import ml_dtypes
import numpy as np

import concourse.bass as bass
import concourse.mybir as mybir
import concourse.tile as tile
from concourse import bacc
from concourse.bass_utils import run_bass_kernel_spmd

N, L, E, H = 4, 2048, 1024, 16
DK = E // H  # 64
NC = 8
BPC = 2  # token blocks per core per batch
TPB = 128  # tokens per block
TPN = BPC * TPB  # 256 tokens per batch per core
TC = N * TPN  # 1024 tokens per core
P = 128
QC = 512  # q' chunk
NQC = 2048 // QC  # 4
NKT = 2048 // P  # 16 key tiles (= s values)
ET = E // P  # 8

F32 = mybir.dt.float32
BF16 = mybir.dt.bfloat16
MM_DT = BF16


def build_nc():
    nc = bacc.Bacc("TRN2", target_bir_lowering=False, debug=False, num_devices=NC)

    xTc = nc.dram_tensor("xTc", [E, TC], MM_DT, kind="ExternalInput").ap()
    wqT = nc.dram_tensor("wqT", [E, E], MM_DT, kind="ExternalInput").ap()
    wkT = nc.dram_tensor("wkT", [E, E], MM_DT, kind="ExternalInput").ap()
    wvT = nc.dram_tensor("wvT", [E, E], MM_DT, kind="ExternalInput").ap()
    woT = nc.dram_tensor("woT", [E, E], MM_DT, kind="ExternalInput").ap()
    outp = nc.dram_tensor("outp", [TC, E], F32, kind="ExternalOutput").ap()

    with tile.TileContext(nc) as tc:
        with (
            tc.tile_pool(name="const", bufs=1) as const,
            tc.tile_pool(name="wpool", bufs=1) as wpool,
            tc.tile_pool(name="xv", bufs=2) as xv_pool,
            tc.tile_pool(name="qk1", bufs=2) as qk1_pool,
            tc.tile_pool(name="expp", bufs=6) as exp_pool,
            tc.tile_pool(name="opt", bufs=2) as opt_pool,
            tc.tile_pool(name="nrm", bufs=2) as nrm_pool,
            tc.tile_pool(name="ops", bufs=2) as op_pool,
            tc.tile_pool(name="scps", bufs=2, space="PSUM") as sc_psum,
            tc.tile_pool(name="pvps", bufs=2, space="PSUM") as pv_psum,
            tc.tile_pool(name="fdps", bufs=2, space="PSUM") as fd_psum,
        ):
            ones_f32 = const.tile([P, P], F32)
            nc.vector.memset(ones_f32[:], 1.0)
            ones_r = const.tile([P, P], mybir.dt.float32r)
            nc.vector.tensor_copy(ones_r[:], ones_f32[:])

            # ---- input DMAs, initiated in consumer-priority order ----
            x_sb = xv_pool.tile([P, ET, TC], MM_DT, tag="xv", name="x_sb")
            xr_ = xTc.rearrange("(a p) t -> p a t", p=P)

            def w_tile(nm):
                return wpool.tile([P, ET, E], MM_DT, tag=nm, name=nm)

            wk_sb, wq_sb, wv_sb, wo_sb = (
                w_tile("wk"), w_tile("wq"), w_tile("wv"), w_tile("wo"))

            def wload(w_sb, w_dram, c0, c1):
                wr = w_dram.rearrange("(a p) d -> p a d", p=P)
                nc.sync.dma_start(
                    out=w_sb[:, :, c0 * P : c1 * P],
                    in_=wr[:, :, c0 * P : c1 * P],
                )

            nc.sync.dma_start(out=x_sb[:, :, 0:TPN], in_=xr_[:, :, 0:TPN])
            wload(wk_sb, wkT, 0, 2)
            wload(wq_sb, wqT, 0, 2)
            wload(wv_sb, wvT, 0, 4)
            wload(wk_sb, wkT, 2, 4)
            wload(wq_sb, wqT, 2, 4)
            wload(wv_sb, wvT, 4, 8)
            wload(wk_sb, wkT, 4, 8)
            wload(wq_sb, wqT, 4, 8)
            for n_ in range(1, N):
                nc.sync.dma_start(
                    out=x_sb[:, :, n_ * TPN : (n_ + 1) * TPN],
                    in_=xr_[:, :, n_ * TPN : (n_ + 1) * TPN],
                )
            wload(wo_sb, woT, 0, 8)

            def project_batch(n):
                """Pieces: (deadline_offset, cost, fn).  cost 1 = ~430ns of
                PE work, 0 = DVE-only.  scores(j) consume k-pair j//4 / q-pair
                u and are emitted BEFORE the drain of their slot (-2 margin);
                PV(j-1) consume v/ones after the drain."""
                v_sb = qk1_pool.tile(
                    [P, BPC, NKT, DK + 1], MM_DT, tag="v", name="v_sb"
                )
                q1t = qk1_pool.tile([P, 2048], MM_DT, tag="q1", name="q1t")
                k1t = qk1_pool.tile([P, 2048], MM_DT, tag="k1", name="k1t")

                def qk_pieces(w_sb, dst, pr_):
                    cell = {}

                    def mk_mm(sub, half):
                        def em():
                            if sub == 0 and half == 0:
                                cell["ps"] = fd_psum.tile(
                                    [P, 2, TPN], F32, tag="fd", name="qkps"
                                )
                            ps = cell["ps"]
                            a2 = 2 * pr_ + sub
                            for a in range(4 * half, 4 * half + 4):
                                nc.tensor.matmul(
                                    ps[:, sub, :],
                                    w_sb[:, a, a2 * P : (a2 + 1) * P],
                                    x_sb[:, a, n * TPN : (n + 1) * TPN],
                                    start=(a == 0),
                                    stop=(a == ET - 1),
                                )
                        return em

                    def evict():
                        ps = cell["ps"]
                        psr = ps.rearrange("q s (b t) -> q s b t", t=TPB)
                        dr = dst.rearrange("q (s t) -> q s t", t=TPB)
                        for rh in range(2):
                            for B in range(BPC):
                                s0 = 4 * pr_ + rh
                                nc.vector.tensor_copy(
                                    dr[B * DK : (B + 1) * DK, s0 : s0 + 3 : 2, :],
                                    psr[rh * DK : (rh + 1) * DK, :, B, :],
                                )

                    return [(1, mk_mm(s, h)) for s in range(2) for h in range(2)] \
                        + [(0, evict)]

                def v_pieces(B, eh):
                    cell = {}
                    tok0 = n * TPN + B * TPB

                    def mk_mm(q):
                        def em():
                            if q == 0:
                                cell["ps"] = fd_psum.tile(
                                    [P, 512], F32, tag="fd", name="vps"
                                )
                            ps = cell["ps"]
                            for a in range(2 * q, 2 * q + 2):
                                nc.tensor.matmul(
                                    ps[:],
                                    x_sb[:, a, tok0 : tok0 + TPB],
                                    wv_sb[:, a, eh * 512 : (eh + 1) * 512],
                                    start=(a == 0),
                                    stop=(a == ET - 1),
                                )
                        return em

                    def evict():
                        nc.vector.tensor_copy(
                            v_sb[:, B, eh * 8 : (eh + 1) * 8, 0:DK],
                            cell["ps"].rearrange("p (s d) -> p s d", d=DK),
                        )

                    return [(1, mk_mm(q)) for q in range(4)] + [(0, evict)]

                def ones_piece():
                    nc.vector.tensor_copy(
                        v_sb[:, :, :, DK], ones_f32[:, 0 : BPC * NKT]
                    )

                pieces = []

                def grp(off, lst):
                    pieces.extend((off, c, f) for c, f in lst)

                grp(-2, qk_pieces(wk_sb, k1t, 0))
                grp(-2, qk_pieces(wq_sb, q1t, 0))
                grp(0, v_pieces(0, 0))
                grp(0, v_pieces(1, 0))
                pieces.append((0, 0, ones_piece))
                grp(2, qk_pieces(wk_sb, k1t, 1))
                grp(6, qk_pieces(wk_sb, k1t, 2))
                grp(7, v_pieces(0, 1))
                grp(7, v_pieces(1, 1))
                grp(10, qk_pieces(wk_sb, k1t, 3))
                grp(14, qk_pieces(wq_sb, q1t, 1))
                grp(31, qk_pieces(wq_sb, q1t, 2))
                grp(48, qk_pieces(wq_sb, q1t, 3))
                return (v_sb, q1t, k1t), pieces

            def make_normalize_piece(opT, rec, B, u):
                def emit():
                    rp = 32 * (2 * (u % 2) + B)
                    bcp = fd_psum.tile([P, QC], F32, tag="fd", name="bcp")
                    nc.tensor.matmul(
                        bcp[:],
                        ones_r[rp : rp + 1, :],
                        rec[u // 2][rp : rp + 1, :],
                        start=True,
                        stop=True,
                        tile_position=(rp, 0),
                    )
                    for sg in range(2):
                        tgt = opT[sg * DK : (sg + 1) * DK,
                                  2 * u : 2 * u + 2, B, :]
                        nc.vector.tensor_mul(
                            tgt,
                            tgt,
                            bcp[sg * DK : (sg + 1) * DK, :].rearrange(
                                "d (sp t) -> d sp t", t=TPB
                            )[:, sg::2, :],
                        )
                return emit

            def make_outproj_pieces(opT, n, B, half):
                cell = {}
                r0 = n * TPN + B * TPB

                def mk_mm(q):
                    def em():
                        if q == 0:
                            cell["ps"] = fd_psum.tile([P, 512], F32,
                                                      tag="fd", name="opps")
                        ps = cell["ps"]
                        for a2 in range(2 * q, 2 * q + 2):
                            nc.tensor.matmul(
                                ps[:],
                                opT[:, a2, B, :],
                                wo_sb[:, a2, half * 512 : (half + 1) * 512],
                                start=(a2 == 0),
                                stop=(a2 == ET - 1),
                            )
                    return em

                def evict():
                    op_sb = op_pool.tile([P, 512], F32, tag="op")
                    nc.vector.tensor_copy(op_sb[:], cell["ps"][:])
                    nc.sync.dma_start(
                        out=outp[r0 : r0 + TPB, half * 512 : (half + 1) * 512],
                        in_=op_sb[:],
                    )

                return [(1, mk_mm(q)) for q in range(4)] + [(0, evict)]

            # ---- deadline-keyed deferred-work queue ----
            SPB = NQC * (NKT + 1)  # drain slots per batch
            feed = []
            seq_counter = [0]

            def push(key, cost, piece):
                bisect.insort(feed, (key, seq_counter[0], cost, piece))
                seq_counter[0] += 1

            def drain(slot, slots_left):
                spent = 0
                while feed and feed[0][0] <= slot:
                    it = feed.pop(0)
                    it[3]()
                    spent += it[2]
                popped = 0
                if feed:
                    want = (len(feed) + slots_left - 1) // max(slots_left, 1)
                    while feed and spent < 1 and popped < max(want, 1) + 1:
                        it = feed.pop(0)
                        it[3]()
                        spent += it[2]
                        popped += 1

            tiles, pieces0 = project_batch(0)
            for off, cost, p in pieces0:
                if off < 0:
                    p()  # batch 0's k0/q0 run eagerly
                else:
                    push(off, cost, p)
            next_state = None

            for n in range(N):
                base = n * SPB
                v_sb, q1t, k1t = tiles
                if n + 1 < N:
                    next_state = project_batch(n + 1)
                    for off, cost, p in next_state[1]:
                        push((n + 1) * SPB + off, cost, p)
                opT = opt_pool.tile([P, ET, BPC, TPB], MM_DT, tag="opT",
                                    name="opT")
                # denominators: tile u//2, partition half u%2, row 32*(2*(u%2)+B)
                sums = [
                    nrm_pool.tile([P, QC], F32, tag="sums", name=f"sums{_i}")
                    for _i in range(2)
                ]
                # only 4 partitions per tile carry data; define the rest so
                # the half-tile reciprocals read initialized memory
                for _i in range(2):
                    nc.vector.memset(sums[_i][:], 1.0)
                rec = [
                    nrm_pool.tile([P, QC], mybir.dt.float32r, tag="rec",
                                  name=f"rec{_i}")
                    for _i in range(2)
                ]
                recf = nrm_pool.tile([P, QC], F32, tag="recf", name="recf")

                for u in range(NQC):
                    qsl = slice(u * QC, (u + 1) * QC)
                    pv = [
                        pv_psum.tile([DK + 1, QC], F32, tag="pv",
                                     name=f"pv{_b}")
                        for _b in range(BPC)
                    ]
                    exps_ring = []
                    for j in range(NKT + 1):
                        if j < NKT:
                            sc = sc_psum.tile([P, BPC, QC], F32, tag="sc")
                            ksl = slice(j * TPB, (j + 1) * TPB)
                            for B in range(BPC):
                                bsl = slice(B * DK, (B + 1) * DK)
                                nc.tensor.matmul(
                                    sc[:, B, :],
                                    k1t[bsl, ksl],
                                    q1t[bsl, qsl],
                                    start=True,
                                    stop=True,
                                )
                            exps = exp_pool.tile([P, BPC, QC], MM_DT,
                                                 tag="exps")
                            nc.scalar.activation(
                                exps[:],
                                sc[:],
                                mybir.ActivationFunctionType.Exp,
                                scale=1.0 / np.sqrt(DK),
                            )
                            exps_ring.append(exps)
                        # deferred work between scores(j) and PV(j-1) hides
                        # the exp latency of tile j-1
                        drain(base + u * (NKT + 1) + j,
                              (NQC - u) * (NKT + 1) - j)
                        if j >= 1:
                            jj = j - 1
                            for B in range(BPC):
                                nc.tensor.matmul(
                                    pv[B][:],
                                    v_sb[:, B, jj, :],
                                    exps_ring[jj][:, B, :],
                                    start=(jj == 0),
                                    stop=(jj == NKT - 1),
                                )
                    hf = u % 2
                    psl = slice(64 * hf, 64 * hf + 64)
                    for B in range(BPC):
                        # unnormalized eviction into opT; s = 4u + sp
                        for sg in range(2):
                            nc.vector.tensor_copy(
                                opT[sg * DK : (sg + 1) * DK,
                                    2 * u : 2 * u + 2, B, :],
                                pv[B][0:DK, :].rearrange(
                                    "d (sp t) -> d sp t", t=TPB
                                )[:, sg::2, :],
                            )
                        rp = 32 * (2 * hf + B)
                        nc.vector.tensor_copy(
                            sums[u // 2][rp : rp + 1, :],
                            pv[B][DK : DK + 1, :],
                        )
                    if hf == 1:
                        # this sums tile (units u-1, u) is complete
                        half2 = u // 2
                        with nc.allow_low_precision(
                            reason="softmax denominators"
                        ):
                            nc.vector.reciprocal_approx_fast(
                                out=recf[:], in_=sums[half2][:]
                            )
                            nc.vector.tensor_copy(rec[half2][:], recf[:])
                        nkey = (base + 40 if u == 1
                                else base + SPB + 8)
                        for uu in (u - 1, u):
                            for B in range(BPC):
                                push(nkey, 1,
                                     make_normalize_piece(opT, rec, B, uu))
                for B in range(BPC):
                    for half in range(2):
                        for cost, p in make_outproj_pieces(opT, n, B, half):
                            push(base + SPB + 20, cost, p)
                if next_state is not None:
                    tiles = next_state[0]

            while feed:
                feed.pop(0)[3]()

    nc.compile()
    return nc


_CACHED_NC = None


def get_nc():
    global _CACHED_NC
    if _CACHED_NC is None:
        _CACHED_NC = build_nc()
    return _CACHED_NC


def make_in_maps(inputs):
    x = np.ascontiguousarray(np.asarray(inputs["x"], dtype=np.float32))
    Wq = np.asarray(inputs["Wq"], dtype=np.float32)
    Wk = np.asarray(inputs["Wk"], dtype=np.float32)
    Wv = np.asarray(inputs["Wv"], dtype=np.float32)
    Wo = np.asarray(inputs["Wo"], dtype=np.float32)

    def cast(a):
        return np.ascontiguousarray(a).astype(ml_dtypes.bfloat16)

    wqT = cast(Wq.T)
    wkT = cast(Wk.T)
    wvT = cast(Wv.T)
    woT = cast(Wo.T)
    xr = x.reshape(N, L, E)

    in_maps = []
    for c in range(NC):
        xc = np.concatenate(
            [xr[n, 256 * c : 256 * (c + 1), :] for n in range(N)], axis=0
        )
        in_maps.append(
            {
                "xTc": cast(xc.T),
                "wqT": wqT,
                "wkT": wkT,
                "wvT": wvT,
                "woT": woT,
            }
        )
    return in_maps


def kernel(x, Wq, Wk, Wv, Wo):
    in_maps = make_in_maps({"x": x, "Wq": Wq, "Wk": Wk, "Wv": Wv, "Wo": Wo})
    res = run_bass_kernel_spmd(get_nc(), in_maps, list(range(NC)))
    out = np.empty((N, L, E), dtype=np.float32)
    for c in range(NC):
        o = res.results[c]["outp"].reshape(N, TPN, E)
        out[:, 256 * c : 256 * (c + 1), :] = o
    return out
